# revision 6
# baseline (speedup 1.0000x reference)
"""CharCNN embedder (ELMo-style) Trainium2 Bass kernel, v4.

v3 (379us) bottlenecks: the DVE drained all conv PSUM via fp32 reduce_max
(220us busy) while the PE repeatedly cooled to a lower p-state in the
resulting gaps (305us busy for ~217us of column work at full clock).

v4 structure:
  - Conv drain split across engines: filter chunks 0..DSET-1 keep the
    direct DVE reduce_max from PSUM (raw max; ReLU+bias applied later per
    token half); chunks DSET..15 are drained by the Scalar engine as a
    fused ReLU+bias PSUM->SBUF bf16 copy and then max-reduced by the DVE
    in its fast 16-bit all-SBUF mode (~4x). relu(max(x)+b)==max(relu(x+b))
    by monotonicity, and the -1e30 indicator rows clamp to 0, which the
    max then ignores exactly like the reference's masked max.
  - h tensors live chan-major as single tiles [128, (mt, k, tok)]; the
    highway uses them directly as matmul stationaries (no PE transposes),
    and the token-major copies for the gating arithmetic are produced by
    DMA xbar transposes, which cost no PE/ACT/DVE time.
  - Highway layer-0 for token half 0 is interleaved into the PE stream
    while the conv's second half still drains, so the PE never idles long
    enough to drop out of its max p-state.
"""

import os
import numpy as np
import ml_dtypes

import concourse.bass as bass
import concourse.mybir as mybir
import concourse.tile as tile
from concourse.bass_utils import run_bass_kernel_spmd

F32 = mybir.dt.float32
BF16 = mybir.dt.bfloat16
NPBF16 = ml_dtypes.bfloat16

CNN_OPTIONS = [(1, 32), (2, 32), (3, 64), (4, 128), (5, 256), (6, 512), (7, 1024)]
EMB_DIM = 16
N_CHARS = 262
MAX_CHARS = 50
N_FILTERS = 2048
OUT_DIM = 512
BATCH, SEQ = 4, 512
NCORES = 8
T_LOC = BATCH * SEQ // NCORES          # 256 tokens per core
COLS = T_LOC * MAX_CHARS               # 12800
COLS_PAD = COLS + 16                   # 12816
KMAX = 7
KROWS = EMB_DIM * KMAX                 # 112
KTOT = KROWS + 6                       # 118 (6 indicator rows for pos 44..49)
NCH = 512                              # xT build chunk width
NXCH = COLS // NCH                     # 25
TOKG = 8                               # tokens per conv matmul
CHUNK_K = [1, 4, 5, 5, 6, 6, 6, 6, 7, 7, 7, 7, 7, 7, 7, 7]
CHUNK_NP = [50 if k == 1 else (MAX_CHARS - k + 1) for k in CHUNK_K]
KC = 16                                # 2048/128 contraction chunks
DSET = 6                               # chunks 0..5 drain via direct DVE reduce
USE_BLOCK_DMAT = True                  # one xbar transpose per token half


def _split_multi_waits(nc):
    """This walrus build encodes at most ONE sync-wait per instruction.
    Hoist extra waits onto dedicated NoOps ahead of the instruction."""
    ctr = [0]
    for f in nc.m.functions:
        for b in f.blocks:
            il = b.instructions
            if not any(
                i.sync_info is not None and len(i.sync_info.on_wait) > 1 for i in il
            ):
                continue
            new = []
            for ins in il:
                si = ins.sync_info
                if si is not None and len(si.on_wait) > 1:
                    waits = list(si.on_wait)
                    for w in waits[:-1]:
                        ctr[0] += 1
                        nop = mybir.InstNoOp(name=f"wsplit-{ctr[0]}", ins=[], outs=[])
                        nop.engine = ins.engine
                        nop.sync_info = mybir.SyncInfo(on_wait=[w], on_update=[])
                        new.append(nop)
                    ins.sync_info = mybir.SyncInfo(
                        on_wait=[waits[-1]], on_update=list(si.on_update)
                    )
                new.append(ins)
            b.instructions = new


def _build_program(split_waits=True):
    nc = bass.Bass(target_bir_lowering=False)

    oh_d = nc.dram_tensor("oh", [384, COLS_PAD], BF16, kind="ExternalInput")
    embt_d = nc.dram_tensor("embt", [384, EMB_DIM], BF16, kind="ExternalInput")
    convw_d = nc.dram_tensor("convw", [KTOT, N_FILTERS], BF16, kind="ExternalInput")
    indic_d = nc.dram_tensor("indic", [6, COLS_PAD], BF16, kind="ExternalInput")
    cbias_d = nc.dram_tensor("cbias", [128, 16], F32, kind="ExternalInput")
    # highway weights: [layer][pass 0..3][k 0..15][128 part][1024 = blocks 2p,2p+1]
    hww_d = [
        nc.dram_tensor(f"hww{l}", [4, KC, 128, 1024], BF16, kind="ExternalInput")
        for l in range(2)
    ]
    hb_d = [
        nc.dram_tensor(f"hb{l}", [1, 2 * N_FILTERS], BF16, kind="ExternalInput")
        for l in range(2)
    ]
    pw_d = nc.dram_tensor("pw", [KC, 128, 512], BF16, kind="ExternalInput")
    pb_d = nc.dram_tensor("pb", [1, 512], BF16, kind="ExternalInput")
    out_d = nc.dram_tensor("outT", [T_LOC, OUT_DIM], F32, kind="ExternalOutput")

    with tile.TileContext(nc) as tc:
        with (
            tc.tile_pool(name="const", bufs=1) as cpool,
            tc.tile_pool(name="scr", bufs=4) as spool,
            tc.tile_pool(name="elem", bufs=9) as epool,
            tc.tile_pool(name="gt", bufs=2) as gtpool,
            tc.tile_pool(name="outp", bufs=2) as outpool,
            tc.tile_pool(name="wgrp", bufs=32) as wpool,
            tc.tile_pool(name="ohp", bufs=4) as ohpool,
        ):
            # ---- persistent SBUF tiles ----
            embt_s = cpool.tile([128, 3 * EMB_DIM], BF16, tag="embt")
            for r in range(3):
                nc.sync.dma_start(
                    embt_s[:, 16 * r : 16 * r + 16], embt_d[128 * r : 128 * r + 128, :]
                )
            cbias_s = cpool.tile([128, 16], F32, tag="cbias")
            nc.sync.dma_start(cbias_s[:], cbias_d[:])
            X7 = cpool.tile([KTOT, COLS_PAD], BF16, tag="X7")
            convw_s = cpool.tile([KTOT, N_FILTERS], BF16, tag="convw")
            hb_s = []
            for l in range(2):
                t = cpool.tile([1, 2 * N_FILTERS], BF16, tag=f"hb{l}")
                nc.sync.dma_start(t[:], hb_d[l][:])
                hb_s.append(t)
            pb_s = cpool.tile([1, 512], BF16, tag="pb")
            nc.sync.dma_start(pb_s[:], pb_d[:])
            onesb_s = cpool.tile([1, 128], BF16, tag="onesb")
            nc.gpsimd.memset(onesb_s[:], 1.0)

            # h tensors, chan-major: cols = 2048*mt + 128*k + t
            hts = cpool.tile([128, 2 * N_FILTERS], BF16, tag="hts")
            h1ts = cpool.tile([128, 2 * N_FILTERS], BF16, tag="h1ts")
            h2ts = cpool.tile([128, 2 * N_FILTERS], BF16, tag="h2ts")
            # token-major: cols = chan (128*k + c)
            htok = [cpool.tile([128, N_FILTERS], BF16, tag=f"htok{t}", name=f"htok{t}") for t in range(2)]
            h1tok = [cpool.tile([128, N_FILTERS], BF16, tag=f"h1tok{t}", name=f"h1tok{t}") for t in range(2)]
            h2tok = [cpool.tile([128, N_FILTERS], BF16, tag=f"h2tok{t}", name=f"h2tok{t}") for t in range(2)]

            def ts_sl(ts_t, k, mt, n=128):
                return ts_t[:, 2048 * mt + 128 * k : 2048 * mt + 128 * k + n]

            # ---- PSUM pools (manual scoping; 8 banks total) ----
            ps_conv_cm = tc.tile_pool(name="ps_conv", bufs=2, space="PSUM")
            ps_conv = ps_conv_cm.__enter__()
            ps_emb_cm = tc.tile_pool(name="ps_emb", bufs=2, space="PSUM")
            ps_emb = ps_emb_cm.__enter__()

            # ---- emb one-hot pipeline ----
            def emb_chunk(c):
                c0 = c * NCH
                oht = ohpool.tile([128, 3 * NCH], BF16, tag="oh", name=f"oh_{c}")
                nc.sync.dma_start(
                    oht[:].rearrange("p (r c) -> p r c", r=3),
                    oh_d[:, c0 : c0 + NCH].rearrange("(r p) c -> p r c", p=128),
                )
                px = ps_emb.tile([16, NCH], F32, space="PSUM", tag="xt", name=f"px{c}")
                for r in range(3):
                    nc.tensor.matmul(
                        px[:],
                        embt_s[:, 16 * r : 16 * r + 16],
                        oht[:, NCH * r : NCH * r + NCH],
                        start=(r == 0),
                        stop=(r == 2),
                    )
                nc.scalar.copy(X7[0:16, c0 : c0 + NCH], px[:])

            def shifts(cl, cu):
                for j in range(1, KMAX):
                    nc.sync.dma_start(
                        X7[16 * j : 16 * j + 16, cl:cu], X7[0:16, cl + j : cu + j]
                    )

            # ---- conv unit with split drain ----
            def conv_unit(nnp, m):
                npos = CHUNK_NP[m]
                mt, tq = nnp // 8, nnp % 8
                ps = ps_conv.tile(
                    [128, 1024], F32, space="PSUM", tag="big", name=f"cv{m}_{nnp}"
                )
                for h in range(2):
                    c0 = (2 * nnp + h) * TOKG * MAX_CHARS
                    rhs = (
                        X7[0:KTOT, c0 : c0 + TOKG * MAX_CHARS]
                        .rearrange("p (t c) -> p t c", c=MAX_CHARS)[:, :, 0:npos]
                    )
                    nc.tensor.matmul(
                        ps[:, 512 * h : 512 * h + TOKG * npos],
                        convw_s[:, 128 * m : 128 * m + 128],
                        rhs,
                        start=True,
                        stop=True,
                    )
                dst = hts[:, 2048 * mt + 128 * m + 16 * tq : 2048 * mt + 128 * m + 16 * tq + 16]
                if m < DSET:
                    ps4 = (
                        ps[:]
                        .rearrange("p (h x) -> p h x", h=2)[:, :, 0 : TOKG * npos]
                        .rearrange("p h (t c) -> p h t c", c=npos)
                    )
                    nc.vector.reduce_max(
                        dst.rearrange("p (h t) -> p h t", h=2),
                        ps4,
                        axis=mybir.AxisListType.X,
                    )
                else:
                    scr = spool.tile([128, 2 * TOKG * 50], BF16, tag="scr", name=f"sc{m}_{nnp}")
                    ps3 = ps[:].rearrange("p (h x) -> p h x", h=2)[:, :, 0 : TOKG * npos]
                    sc3 = scr[:, 0 : 2 * TOKG * npos].rearrange("p (h x) -> p h x", h=2)
                    nc.scalar.activation(
                        sc3, ps3, mybir.ActivationFunctionType.Relu,
                        bias=cbias_s[:, m : m + 1], scale=1.0,
                    )
                    nc.vector.reduce_max(
                        dst,
                        scr[:, 0 : 2 * TOKG * npos].rearrange("p (t c) -> p t c", c=npos),
                        axis=mybir.AxisListType.X,
                    )

            def post_relu(mt):
                for m in range(DSET):
                    hsl = ts_sl(hts, m, mt)
                    nc.scalar.activation(
                        hsl, hsl, mybir.ActivationFunctionType.Relu,
                        bias=cbias_s[:, m : m + 1], scale=1.0,
                    )

            def htok_transposes(mt):
                if USE_BLOCK_DMAT:
                    nc.sync.dma_start_transpose(
                        htok[mt][:].rearrange("p (k c) -> p k c", c=128),
                        hts[:, 2048 * mt : 2048 * mt + 2048],
                    )
                else:
                    for k in range(KC):
                        nc.sync.dma_start_transpose(
                            htok[mt][:, 128 * k : 128 * k + 128],
                            ts_sl(hts, k, mt),
                        )

            # ---- highway machinery (token-major out, chan-chunk stationary) ----
            ps_hw_pools = []

            def hw_ps(nm):
                pool = ps_hw_pools[hw_ps.idx % len(ps_hw_pools)]
                hw_ps.idx += 1
                return pool.tile([128, 512], F32, space="PSUM", tag="hwp", name=nm)

            hw_ps.idx = 0

            def wg_dma(l, p):
                grps = []
                for k in range(KC):
                    g = wpool.tile([128, 1024], BF16, tag="wg", name=f"wg{l}{p}k{k}")
                    nc.sync.dma_start(g[:], hww_d[l][p, k])
                    grps.append(g)
                return grps

            rg = {}

            def hw_pass_closures(l, p, mt, src_ts, grps):
                """Closures for one (layer, pass, token-half): 2 psum chains."""
                cls = []
                psA = [None]

                def start():
                    psA[0] = (hw_ps(f"A{l}{p}{mt}"), hw_ps(f"B{l}{p}{mt}"))

                cls.append(start)

                def mk_mm(k):
                    def f():
                        st = ts_sl(src_ts, k, mt)
                        g = grps[k] if not callable(grps) else grps(k)
                        nc.tensor.matmul(
                            psA[0][0][:], st, g[:, 0:512],
                            start=(k == 0), stop=False,
                        )
                        nc.tensor.matmul(
                            psA[0][1][:], st, g[:, 512:1024],
                            start=(k == 0), stop=False,
                        )
                    return f

                for k in range(KC):
                    cls.append(mk_mm(k))

                def bias_acts():
                    for i in range(2):
                        b = 2 * p + i
                        nc.tensor.matmul(
                            psA[0][i][:], onesb_s[0:1, :],
                            hb_s[l][0:1, 512 * b : 512 * b + 512],
                            start=False, stop=True,
                        )
                    for i in range(2):
                        b = 2 * p + i
                        t = epool.tile([128, 512], BF16, tag="rg", name=f"rg{l}{p}{mt}{i}")
                        fn = (
                            mybir.ActivationFunctionType.Relu
                            if p < 2
                            else mybir.ActivationFunctionType.Sigmoid
                        )
                        nc.scalar.activation(t[:], psA[0][i][:], fn)
                        rg[(l, mt, b)] = t
                cls.append(bias_acts)
                return cls

            def gating_closures(l, mt, i, src_tok, dst_tok, dst_ts):
                """Block i (512 chans): h' = g*h + (1-g)*r, then transposes."""
                cls = []

                def gate():
                    r = rg[(l, mt, i)]
                    g = rg[(l, mt, 4 + i)]
                    hsl = src_tok[mt][:, 512 * i : 512 * i + 512]
                    t1 = gtpool.tile([128, 512], BF16, tag="gt1", name=f"t1_{l}{mt}{i}")
                    nc.vector.tensor_tensor(
                        out=t1[:], in0=hsl, in1=r[:], op=mybir.AluOpType.subtract
                    )
                    t2 = gtpool.tile([128, 512], BF16, tag="gt2", name=f"t2_{l}{mt}{i}")
                    nc.vector.tensor_tensor(
                        out=t2[:], in0=g[:], in1=t1[:], op=mybir.AluOpType.mult
                    )
                    dsl = dst_tok[mt][:, 512 * i : 512 * i + 512]
                    nc.vector.tensor_tensor(
                        out=dsl, in0=t2[:], in1=r[:], op=mybir.AluOpType.add
                    )
                cls.append(gate)

                if i % 2 == 1:
                    # after blocks (i-1, i): transpose that 1024-chan half back
                    def tposes():
                        i0 = i - 1
                        if USE_BLOCK_DMAT:
                            nc.sync.dma_start_transpose(
                                hts_part_3d(dst_ts, mt, 8 * (i0 // 2)),
                                dst_tok[mt][:, 512 * i0 : 512 * i0 + 1024],
                            )
                        else:
                            for k in range(4 * i0, 4 * i0 + 8):
                                nc.sync.dma_start_transpose(
                                    ts_sl(dst_ts, k, mt),
                                    dst_tok[mt][:, 128 * k : 128 * k + 128],
                                )
                    cls.append(tposes)
                return cls

            def hts_part_3d(ts_t, mt, k0):
                # [128, 8, 128] view over ts cols 2048*mt+128*k0 .. +1024
                return (
                    ts_t[:, 2048 * mt + 128 * k0 : 2048 * mt + 128 * k0 + 1024]
                    .rearrange("p (k t) -> p k t", t=128)
                )

            # ================= schedule =================
            # ---- phase 1: conv tokens 0..127 (nnp 0..7) + emb pump ----
            for c in range(3):
                emb_chunk(c)
            nc.sync.dma_start(convw_s[:], convw_d[:])
            nc.sync.dma_start(X7[112:118, :], indic_d[:])
            nc.gpsimd.memset(X7[0:16, COLS:COLS_PAD], 0.0)
            emb_chunk(3)
            shifts(0, 1610)
            for c in range(4, 9):
                emb_chunk(c)
            shifts(1610, 3578)

            next_c = [9]
            SH = {13: (3578, 6650), 19: (6650, 9722), 25: (9722, COLS + 6)}

            def pump_emb():
                if next_c[0] >= NXCH:
                    return False
                c = next_c[0]
                emb_chunk(c)
                next_c[0] += 1
                if c + 1 in SH:
                    shifts(*SH[c + 1])
                return True

            grp_cache = {}
            u = 0
            for nnp in range(8):
                for m in range(16):
                    conv_unit(nnp, m)
                    u += 1
                    if u % 6 == 0:
                        if not pump_emb():
                            if (0, 0) not in grp_cache:
                                grp_cache[(0, 0)] = wg_dma(0, 0)
                            elif (0, 1) not in grp_cache:
                                grp_cache[(0, 1)] = wg_dma(0, 1)
            while pump_emb():
                pass
            for p in (0, 1):
                if (0, p) not in grp_cache:
                    grp_cache[(0, p)] = wg_dma(0, p)
            ps_emb_cm.__exit__(None, None, None)

            ps_hwA_cm = tc.tile_pool(name="ps_hwA", bufs=4, space="PSUM")
            ps_hw_pools.append(ps_hwA_cm.__enter__())

            # ---- phase 2: conv tokens 128..255 + highway L0 mt0 interleaved ----
            feed = []
            feed.append(lambda: post_relu(0))
            feed.append(lambda: htok_transposes(0))
            for p in range(4):
                cls = hw_pass_closures(0, p, 0, hts, (lambda pp: (lambda k: grp_cache[(0, pp)][k]))(p))
                if p + 2 <= 3:
                    def mk_dma(pp):
                        def f():
                            grp_cache[(0, pp)] = wg_dma(0, pp)
                        return f
                    cls.insert(1, mk_dma(p + 2))
                feed.extend(cls)
                if p >= 2:
                    for i in (2 * (p - 2), 2 * (p - 2) + 1):
                        feed.extend(gating_closures(0, 0, i, htok, h1tok, h1ts))
            # prefetch the layer-0 re-fetch for token half 1 (passes 0..1)
            regrp = {}
            for p in (0, 1):
                def mk_re(pp):
                    def f():
                        regrp[pp] = wg_dma(0, pp)
                    return f
                feed.append(mk_re(p))

            fi = [0]

            def pop_feed():
                if fi[0] < len(feed):
                    feed[fi[0]]()
                    fi[0] += 1

            for nnp in range(8, 16):
                for m in range(16):
                    conv_unit(nnp, m)
                    pop_feed()
            while fi[0] < len(feed):
                pop_feed()

            ps_hwA_cm.__exit__(None, None, None)
            ps_conv_cm.__exit__(None, None, None)
            ps_hw_pools.clear()
            ps_hwA2_cm = tc.tile_pool(name="ps_hwA2", bufs=4, space="PSUM")
            ps_hw_pools.append(ps_hwA2_cm.__enter__())
            ps_hwB_cm = tc.tile_pool(name="ps_hwB", bufs=4, space="PSUM")
            ps_hw_pools.append(ps_hwB_cm.__enter__())

            # ---- phase 3: highway L0 mt1 (weight groups re-fetched) ----
            post_relu(1)
            htok_transposes(1)
            l1grp = {}
            for p in range(4):
                if p + 2 <= 3:
                    regrp[p + 2] = wg_dma(0, p + 2)
                else:
                    l1grp[p - 2] = wg_dma(1, p - 2)
                for f in hw_pass_closures(0, p, 1, hts, regrp[p]):
                    f()
                if p >= 2:
                    for i in (2 * (p - 2), 2 * (p - 2) + 1):
                        for f in gating_closures(0, 1, i, htok, h1tok, h1ts):
                            f()

            # ---- phase 4: highway L1, both token halves per pass ----
            for p in range(4):
                if p + 2 <= 3:
                    l1grp[p + 2] = wg_dma(1, p + 2)
                for mt in (0, 1):
                    for f in hw_pass_closures(1, p, mt, h1ts, l1grp[p]):
                        f()
                if p == 1:
                    # prefetch projection slabs
                    pgrps = []
                    for j in range(8):
                        pg = wpool.tile([128, 1024], BF16, tag="wg", name=f"pgrp{j}")
                        nc.sync.dma_start(
                            pg[:].rearrange("p (k x) -> p k x", k=2),
                            pw_d[2 * j : 2 * j + 2].rearrange("k p x -> p k x"),
                        )
                        pgrps.append(pg)
                if p >= 2:
                    for mt in (0, 1):
                        for i in (2 * (p - 2), 2 * (p - 2) + 1):
                            for f in gating_closures(1, mt, i, h1tok, h2tok, h2ts):
                                f()

            # ---- phase 5: projection ----
            for mt in range(2):
                ps = hw_ps(f"pj{mt}")
                for k in range(KC):
                    nc.tensor.matmul(
                        ps[:],
                        ts_sl(h2ts, k, mt),
                        pgrps[k // 2][:, 512 * (k % 2) : 512 * (k % 2) + 512],
                        start=(k == 0), stop=False,
                    )
                nc.tensor.matmul(
                    ps[:], onesb_s[0:1, :], pb_s[0:1, :], start=False, stop=True
                )
                oc = outpool.tile([128, 512], F32, tag="out", name=f"oc{mt}")
                nc.scalar.copy(oc[:], ps[:])
                nc.sync.dma_start(out_d[128 * mt : 128 * mt + 128, :], oc[:])

            ps_hwB_cm.__exit__(None, None, None)
            ps_hwA2_cm.__exit__(None, None, None)

    if split_waits:
        _split_multi_waits(nc)
    return nc


def _prep_weights(inputs):
    conv_ws = [np.asarray(inputs[f"conv_w{i}"], np.float32) for i in range(7)]
    conv_bs = [np.asarray(inputs[f"conv_b{i}"], np.float32) for i in range(7)]

    W7 = np.zeros((KTOT, N_FILTERS), np.float32)
    o0 = 0
    for (ksz, oc), w in zip(CNN_OPTIONS, conv_ws):
        for j in range(ksz):
            W7[16 * j : 16 * j + 16, o0 : o0 + oc] = w[:, :, j].T
        for i in range(6):
            if (44 + i) > (MAX_CHARS - ksz):
                W7[KROWS + i, o0 : o0 + oc] = -1e30
        o0 += oc

    b_all = np.concatenate(conv_bs)
    cbias = b_all.reshape(16, 128).T.astype(np.float32)

    indic = np.zeros((6, COLS_PAD), np.float32)
    for i in range(6):
        indic[i, (44 + i) : COLS : MAX_CHARS] = 1.0

    emb = np.asarray(inputs["emb"], np.float32)
    embt = np.zeros((384, EMB_DIM), np.float32)
    embt[:N_CHARS] = emb

    out = {
        "embt": embt.astype(NPBF16),
        "convw": W7.astype(NPBF16),
        "indic": indic.astype(NPBF16),
        "cbias": cbias,
        "pb": np.asarray(inputs["proj_b"], np.float32)[None, :].astype(NPBF16),
    }
    for l in range(2):
        w = np.asarray(inputs[f"hw_w{l}"], np.float32)   # [4096, 2048]
        wt = np.ascontiguousarray(w.T)                    # [2048 in, 4096 out]
        arr = wt.reshape(KC, 128, 4, 1024).transpose(2, 0, 1, 3)
        out[f"hww{l}"] = np.ascontiguousarray(arr).astype(NPBF16)
        out[f"hb{l}"] = np.asarray(inputs[f"hw_b{l}"], np.float32)[None, :].astype(NPBF16)
    pwt = np.asarray(inputs["proj_w"], np.float32).T      # [2048, 512]
    out["pw"] = np.ascontiguousarray(pwt.reshape(KC, 128, 512)).astype(NPBF16)
    return out


_NC_CACHE = []
LAST_RESULT = {}


def kernel(**inputs) -> np.ndarray:
    if not _NC_CACHE:
        _NC_CACHE.append(_build_program())
    nc = _NC_CACHE[0]

    shared = _prep_weights(inputs)
    ids = np.asarray(inputs["batch_ids"]).astype(np.int64).reshape(-1, MAX_CHARS)
    rng384 = np.arange(384, dtype=np.int64)
    in_maps = []
    for core in range(NCORES):
        flat = ids[core * T_LOC : (core + 1) * T_LOC].reshape(-1)
        oh = np.zeros((384, COLS_PAD), NPBF16)
        oh[:, :COLS] = (flat[None, :] == rng384[:, None]).astype(NPBF16)
        in_maps.append({"oh": oh, **shared})

    trace = bool(int(os.environ.get("KERNEL_TRACE", "0")))
    res = run_bass_kernel_spmd(
        nc, in_maps, core_ids=list(range(NCORES)), trace=trace
    )
    LAST_RESULT["exec_time_ns"] = res.exec_time_ns
    LAST_RESULT["trace"] = res.instructions_and_trace

    parts = [res.results[c]["outT"] for c in range(NCORES)]  # each [256, 512]
    out = np.concatenate(parts, axis=0).reshape(BATCH, SEQ, OUT_DIM)
    return np.ascontiguousarray(out.astype(np.float32))
